# revision 27
# baseline (speedup 1.0000x reference)
"""Trainium2 Bass kernel for CorrectedPartialCharges.

out[i] = pc[i] + (total_charge[g] - seg_sum[g]) / n_atoms[g],  g = i // 256

Sharding: graphs are data-parallel across the 8 cores (4096 graphs /
1,048,576 atoms per core); segment sums and the gather-broadcast stay
device-local. On each core, partition p owns 32 contiguous graphs, so a
per-graph segment sum is a free-axis reduce over [128, K, 256] tiles and
the correction is a broadcast add.

Schedule (HBM-bound, ~23.3us of DMA at 360 GB/s/core is the floor):
- All input dma_starts are issued up front on the SP (sync) HWDGE ring,
  followed by the output dma_starts in compute order on the same ring.
  Ring FIFO then gives inputs strict priority and the 16 DMA engines
  never idle between the input and output streams.
- total_charge / n_atoms are pre-divided on the host and packed into a
  single [128, 64] constant tensor loaded first on the SP ring: the tiny
  transfer wakes the DGE early and lands well before first use. A dummy
  ACT op at kernel start pulls the one-time ACT_TABLE_LOAD off the
  critical path.
- Per tile: segment reduce on DVE only (free-axis reduce is DVE-only,
  and its ~1.34 ns/elem rate is the tightest pipeline stage), the tiny
  leftover math on Pool, then the broadcast add split between Pool and
  ACT (DVE+ACT on the last tile, when DVE has no more reduces).
"""

import numpy as np

import concourse.bacc as bacc
import concourse.bass as bass
import concourse.mybir as mybir
import concourse.tile as tile
from concourse.bass_utils import run_bass_kernel_spmd

N_CORES = 8
ATOMS_PER_GRAPH = 256
N_GRAPHS = 32768
N_ATOMS = N_GRAPHS * ATOMS_PER_GRAPH
P = 128

G_PER_CORE = N_GRAPHS // N_CORES          # 4096 graphs per core
A_PER_CORE = G_PER_CORE * ATOMS_PER_GRAPH  # 1,048,576 atoms per core
GP = G_PER_CORE // P                       # 32 graphs per partition
AP_FREE = A_PER_CORE // P                  # 8192 atoms per partition

# Knobs read by test.py when experimenting.
# Tile widths (atoms per partition per tile). Mostly-uniform 2048 keeps the
# DVE reduce (the slowest pipeline stage) matched to the DMA input period;
# the tapered tail shortens the last tile's serial chain and the final
# straggler queue's share.
WIDTHS = (2048, 2048, 2048, 1536, 512)
# Per-tile (pool_blocks, act_blocks) split of the k broadcast-add blocks;
# remaining blocks (if any) go to DVE. None = derived below.
SPLITS = None

_TRACE = False
_TRACE_KWARGS = {}


def _bcast(left_view, reps):
    """[P, k] AP -> [P, k, reps] AP with stride-0 inner dim."""
    return bass.AP(
        left_view.tensor,
        left_view.offset,
        [list(left_view.ap[0]), list(left_view.ap[1]), [0, reps]],
    )


def _build(widths=WIDTHS, splits=SPLITS, cst_ring="sync", layout="strided",
           kick=False, rings="sp"):
    # rings: "sp" = all bulk IO on the SP ring; "dual" = inputs alternate
    # SP/ACT rings, each output on the opposite ring from its input (per-ring
    # FIFO still puts all of a ring's inputs ahead of its outputs); "out_act"
    # = inputs on SP, outputs on ACT.
    nt = len(widths)
    assert sum(widths) == AP_FREE
    ks = [w // ATOMS_PER_GRAPH for w in widths]
    assert all(w % ATOMS_PER_GRAPH == 0 for w in widths)
    if splits is None:
        # Last tile: DVE (done reducing by then) + ACT. Others: Pool + ACT,
        # with ACT taking the bigger share (Pool adds run at ~0.42 of
        # roofline; Pool also does the per-tile leftover math).
        splits = []
        for t, k in enumerate(ks):
            if t == nt - 1:
                splits.append((0, (k + 1) // 2))
            else:
                splits.append((k * 3 // 8, k - k * 3 // 8))

    nc = bacc.Bacc(None, target_bir_lowering=False)

    pc = nc.dram_tensor("pc", [A_PER_CORE], mybir.dt.float32, kind="ExternalInput")
    # cst row p = [tch[p*GP:(p+1)*GP] / nat, 1 / nat] precomputed on host.
    cst = nc.dram_tensor("cst", [P * 2 * GP], mybir.dt.float32, kind="ExternalInput")
    out = nc.dram_tensor("out", [A_PER_CORE], mybir.dt.float32, kind="ExternalOutput")

    pc_v = pc[:].rearrange("(p n) -> p n", p=P)
    out_v = out[:].rearrange("(p n) -> p n", p=P)
    cst_v = cst[:].rearrange("(p n) -> p n", p=P)

    def in_view(t, offs):
        if layout == "strided":
            return pc_v[:, offs[t] : offs[t + 1]]
        return pc[:][P * offs[t] : P * offs[t + 1]].rearrange("(p n) -> p n", p=P)

    def out_view(t, offs):
        if layout == "strided":
            return out_v[:, offs[t] : offs[t + 1]]
        return out[:][P * offs[t] : P * offs[t + 1]].rearrange("(p n) -> p n", p=P)

    with tile.TileContext(nc) as tc:
        with (
            tc.tile_pool(name="io", bufs=nt) as io_pool,
            tc.tile_pool(name="small", bufs=2 * nt + 1) as small_pool,
        ):
            # Consts first on the SP ring: the tiny transfer wakes the DGE
            # ~1us before the first bulk input descriptor is ready, and the
            # 32KB land well before first use.
            if kick:
                dummy = small_pool.tile([1, 1], mybir.dt.float32, tag="dummy")
                nc.sync.dma_start(out=dummy[:], in_=cst_v[0:1, 0:1])
            cst_t = small_pool.tile([P, 2 * GP], mybir.dt.float32, tag="cst")
            cst_eng = nc.sync if cst_ring == "sync" else nc.scalar
            cst_eng.dma_start(out=cst_t[:], in_=cst_v)
            tcn = cst_t[:, 0:GP]       # total_charge / n_atoms
            rna = cst_t[:, GP:2 * GP]  # 1 / n_atoms

            # Dummy ACT op so the one-time ACT_TABLE_LOAD runs at kernel
            # start, off the critical path.
            warm = small_pool.tile([P, 1], mybir.dt.float32, tag="warm")
            nc.gpsimd.memset(warm[:], 0.0)
            nc.scalar.add(out=warm[:], in_=warm[:], add=0.0)

            # All input DMAs up front on the SP ring.
            offs = [0]
            for w in widths:
                offs.append(offs[-1] + w)
            goffs = [o // ATOMS_PER_GRAPH for o in offs]

            def in_eng(t):
                if rings == "dual":
                    return nc.sync if t % 2 == 0 else nc.scalar
                return nc.sync

            def out_eng(t):
                if rings == "dual":
                    return nc.scalar if t % 2 == 0 else nc.sync
                return nc.scalar if rings == "out_act" else nc.sync

            xs = []
            for t, w in enumerate(widths):
                x = io_pool.tile([P, w], mybir.dt.float32, tag=f"x{t}")
                in_eng(t).dma_start(out=x[:], in_=in_view(t, offs))
                xs.append(x)

            for t, w in enumerate(widths):
                x = xs[t]
                k = ks[t]
                pb, ab = splits[t]
                x3 = x[:].rearrange("p (k a) -> p k a", a=ATOMS_PER_GRAPH)

                seg = small_pool.tile([P, k], mybir.dt.float32, tag="seg")
                nc.vector.reduce_sum(out=seg[:], in_=x3, axis=mybir.AxisListType.X)

                # left = tcn - seg * rna, off DVE so the reduce stream
                # keeps pace with the DMA input stream.
                lr_eng = nc.vector if t == nt - 1 else nc.gpsimd
                left = small_pool.tile([P, k], mybir.dt.float32, tag="left")
                lr_eng.tensor_mul(
                    out=left[:], in0=seg[:], in1=rna[:, goffs[t] : goffs[t] + k]
                )
                lr_eng.tensor_sub(
                    out=left[:], in0=tcn[:, goffs[t] : goffs[t] + k], in1=left[:]
                )

                # Broadcast add, split across engines by 256-atom block:
                # Pool gets [0, pb), ACT gets [pb, pb+ab), DVE the rest.
                if pb > 0:
                    lo = x[:, : pb * ATOMS_PER_GRAPH].rearrange(
                        "p (k a) -> p k a", a=ATOMS_PER_GRAPH
                    )
                    nc.gpsimd.tensor_add(
                        out=lo, in0=lo, in1=_bcast(left[:, :pb], ATOMS_PER_GRAPH)
                    )
                for j in range(pb, pb + ab):
                    blk = x[:, j * ATOMS_PER_GRAPH : (j + 1) * ATOMS_PER_GRAPH]
                    nc.scalar.add(out=blk, in_=blk, add=left[:, j : j + 1])
                if pb + ab < k:
                    hi = x[:, (pb + ab) * ATOMS_PER_GRAPH :].rearrange(
                        "p (k a) -> p k a", a=ATOMS_PER_GRAPH
                    )
                    nc.vector.tensor_add(
                        out=hi,
                        in0=hi,
                        in1=_bcast(left[:, pb + ab :], ATOMS_PER_GRAPH),
                    )

                # Output DMA: queues drain it right after the pending input
                # descriptors of its ring, FIFO, no idle.
                out_eng(t).dma_start(out=out_view(t, offs), in_=x[:])

    nc.finalize()
    return nc


_NC_CACHE = {}


def _get_nc(widths=None, splits=None, cst_ring="sync", layout="strided",
            kick=False, rings="sp"):
    if widths is None:
        widths = WIDTHS
    if splits is None:
        splits = SPLITS
    key = (tuple(widths), tuple(splits) if splits else None, cst_ring, layout,
           kick, rings)
    if key not in _NC_CACHE:
        _NC_CACHE[key] = _build(widths, splits, cst_ring, layout, kick, rings)
    return _NC_CACHE[key]


def _prepare_in_maps(pc, total_charge, n_atoms, layout="strided", widths=WIDTHS):
    naf = n_atoms.astype(np.float32)
    tcn_full = (total_charge / naf).reshape(N_CORES, G_PER_CORE)
    rna_full = (1.0 / naf).reshape(N_CORES, G_PER_CORE)
    if layout == "strided":
        # partition p owns graphs [p*GP, (p+1)*GP)
        tcn = tcn_full.reshape(N_CORES, P, GP)
        rna = rna_full.reshape(N_CORES, P, GP)
    else:
        # contig: tile t covers graphs [128*goffs[t], 128*goffs[t+1]),
        # partition p owns k_t consecutive graphs within the tile block.
        ks = [w // ATOMS_PER_GRAPH for w in widths]
        tcn = np.empty((N_CORES, P, GP), dtype=np.float32)
        rna = np.empty((N_CORES, P, GP), dtype=np.float32)
        go = 0
        for k in ks:
            blk = slice(P * go, P * (go + k))
            tcn[:, :, go : go + k] = tcn_full[:, blk].reshape(N_CORES, P, k)
            rna[:, :, go : go + k] = rna_full[:, blk].reshape(N_CORES, P, k)
            go += k
    cst = np.ascontiguousarray(
        np.concatenate([tcn, rna], axis=2).reshape(N_CORES, -1)
    )
    return [
        {
            "pc": pc[c * A_PER_CORE : (c + 1) * A_PER_CORE],
            "cst": cst[c],
        }
        for c in range(N_CORES)
    ]


def _cpu_fallback(pc, total_charge, batch, n_atoms):
    num_segments = n_atoms.shape[0]
    seg = np.bincount(batch, weights=pc.astype(np.float64), minlength=num_segments)
    leftover = (total_charge - seg.astype(np.float32)) / n_atoms.astype(np.float32)
    return (pc + leftover[batch]).astype(np.float32)


def kernel(**inputs) -> np.ndarray:
    pc = np.ascontiguousarray(
        np.asarray(inputs["node_outputs"], dtype=np.float32).reshape(-1)
    )
    total_charge = np.ascontiguousarray(
        np.asarray(inputs["total_charge"], dtype=np.float32).reshape(-1)
    )
    batch = np.asarray(inputs["batch"]).reshape(-1)
    n_atoms = np.ascontiguousarray(np.asarray(inputs["n_atoms"], dtype=np.int32).reshape(-1))

    # The device kernel hardcodes the uniform 256-atoms-per-graph layout the
    # reference generator produces; anything else goes through numpy.
    if (
        pc.shape[0] != N_ATOMS
        or total_charge.shape[0] != N_GRAPHS
        or not np.array_equal(
            batch.astype(np.int64),
            np.arange(N_ATOMS, dtype=np.int64) // ATOMS_PER_GRAPH,
        )
    ):
        return _cpu_fallback(pc, total_charge, batch, n_atoms)

    nc = _get_nc()
    in_maps = _prepare_in_maps(pc, total_charge, n_atoms)
    res = run_bass_kernel_spmd(
        nc, in_maps, list(range(N_CORES)), trace=_TRACE, **_TRACE_KWARGS
    )
    out = np.concatenate([r["out"] for r in res.results])
    if _TRACE:
        kernel.last_results = res
    return out
